# revision 1
# baseline (speedup 1.0000x reference)
"""GroupConvMLP Trainium2 kernel.

Problem: x [16384, 128] -> per-group MLP, G=8 groups of S=16 channels.
Per group g: h = x_g @ W1_g.T + b1; swish; @W2.T+b2; swish; @W3.T+b3;
swish; @W4.T+b4.  swish(x) = x*sigmoid(x*softplus(beta))/1.1.

Sharding: group-parallel, one group per NeuronCore (8 groups, 8 cores).

Device kernel (per core) is a pure matmul+Silu chain in channel-major
layout [C, B]:
  - host folds softplus(beta) and the 1/1.1 factor into the weights:
      h' = sp*h  =>  W' = sp*W, b' = sp*b and
      a = h*sigmoid(h*sp)/1.1 = silu(h')/(1.1*sp) = silu(h') * c
    with c folded into the NEXT layer's weights.
  - matmuls run as float32r (TF32-like, 1 cycle/row at N>=256).
  - activations on ScalarE: Silu(psum + bias) in one instruction.
  - B=16384 processed in 32 tiles of N=512 columns.
"""
import numpy as np

import concourse.bass as bass
import concourse.mybir as mybir
from concourse.bass_utils import run_bass_kernel_spmd

B = 16384
G = 8
S = 16
H = 512
SWISH_DIV = 1.1

N = 512            # batch columns per tile
T = B // N         # 32 tiles
KC = H // 128      # 4 contraction chunks for H-dim
OC = H // 128      # 4 output chunks for H-dim
NBX = 4            # x tile buffers
NBY = 4            # y tile buffers
LA = 2             # DMA lookahead (in-DMAs run LA tiles ahead of out-DMAs)

F32 = mybir.dt.float32
F32R = mybir.dt.float32r

_CACHED = {}


def build_nc():
    nc = bass.Bass()

    xT = nc.declare_dram_parameter("xT", [S, B], F32, isOutput=False)
    w1 = nc.declare_dram_parameter("w1", [S, H], F32, isOutput=False)
    w2 = nc.declare_dram_parameter("w2", [H, H], F32, isOutput=False)
    w3 = nc.declare_dram_parameter("w3", [H, H], F32, isOutput=False)
    w4 = nc.declare_dram_parameter("w4", [H, S], F32, isOutput=False)
    b1 = nc.declare_dram_parameter("b1", [128, OC], F32, isOutput=False)
    b2 = nc.declare_dram_parameter("b2", [128, OC], F32, isOutput=False)
    b3 = nc.declare_dram_parameter("b3", [128, OC], F32, isOutput=False)
    b4 = nc.declare_dram_parameter("b4", [S, 1], F32, isOutput=False)
    y = nc.declare_dram_parameter("y", [S, B], F32, isOutput=True)

    # SBUF
    w1s = nc.alloc_sbuf_tensor("w1s", [S, H], F32R)
    w2s = nc.alloc_sbuf_tensor("w2s", [128, KC * H], F32R)   # chunk (k,o) at 512k+128o
    w3s = nc.alloc_sbuf_tensor("w3s", [128, KC * H], F32R)
    w4s = nc.alloc_sbuf_tensor("w4s", [128, KC * S], F32R)   # chunk k at 16k
    b1s = nc.alloc_sbuf_tensor("b1s", [128, OC], F32)
    b2s = nc.alloc_sbuf_tensor("b2s", [128, OC], F32)
    b3s = nc.alloc_sbuf_tensor("b3s", [128, OC], F32)
    b4s = nc.alloc_sbuf_tensor("b4s", [S, 1], F32)
    xs = [nc.alloc_sbuf_tensor(f"xs{i}", [S, N], F32R) for i in range(NBX)]
    ys = [nc.alloc_sbuf_tensor(f"ys{i}", [S, N], F32) for i in range(NBY)]
    a1 = nc.alloc_sbuf_tensor("a1", [128, KC * N], F32R)     # k-chunk at 512k
    a2 = nc.alloc_sbuf_tensor("a2", [128, KC * N], F32R)
    a3 = nc.alloc_sbuf_tensor("a3", [128, KC * N], F32R)

    # PSUM: 8 banks
    ps1 = [nc.alloc_psum_tensor(f"ps1_{i}", [128, N], F32) for i in range(4)]
    ps2 = [nc.alloc_psum_tensor(f"ps2_{i}", [128, N], F32) for i in range(4)]

    n_wdma = 17  # weight DMAs emitted below

    with (
        nc.Block() as block,
        nc.semaphore("dma_in") as dma_in,
        nc.semaphore("dma_out") as dma_out,
        nc.semaphore("mm_sem") as mm_sem,
        nc.semaphore("act_sem") as act_sem,
        nc.semaphore("dve_sem") as dve_sem,
    ):
        @block.sync
        def _(sync):
            r = F32R
            sync.dma_start(out=w1s.ap(), in_=w1.ap().bitcast(r)).then_inc(dma_in, 16)
            for k in range(KC):
                sync.dma_start(
                    out=w2s.ap()[:, k * H:(k + 1) * H],
                    in_=w2.ap()[128 * k:128 * (k + 1), :].bitcast(r),
                ).then_inc(dma_in, 16)
            for k in range(KC):
                sync.dma_start(
                    out=w3s.ap()[:, k * H:(k + 1) * H],
                    in_=w3.ap()[128 * k:128 * (k + 1), :].bitcast(r),
                ).then_inc(dma_in, 16)
            for k in range(KC):
                sync.dma_start(
                    out=w4s.ap()[:, k * S:(k + 1) * S],
                    in_=w4.ap()[128 * k:128 * (k + 1), :].bitcast(r),
                ).then_inc(dma_in, 16)
            sync.dma_start(out=b1s.ap(), in_=b1.ap()).then_inc(dma_in, 16)
            sync.dma_start(out=b2s.ap(), in_=b2.ap()).then_inc(dma_in, 16)
            sync.dma_start(out=b3s.ap(), in_=b3.ap()).then_inc(dma_in, 16)
            sync.dma_start(out=b4s.ap(), in_=b4.ap()).then_inc(dma_in, 16)

            for i in range(T + LA):
                if i < T:
                    if i >= NBX:
                        # xs[i % NBX] still being read by PE tile i-NBX L1
                        sync.wait_ge(mm_sem, 13 * (i - NBX) + 4)
                    sync.dma_start(
                        out=xs[i % NBX].ap(),
                        in_=xT.ap()[:, N * i:N * (i + 1)].bitcast(r),
                    ).then_inc(dma_in, 16)
                if i >= LA:
                    j = i - LA
                    sync.wait_ge(dve_sem, j + 1)
                    sync.dma_start(
                        out=y.ap()[:, N * j:N * (j + 1)],
                        in_=ys[j % NBY].ap(),
                    ).then_inc(dma_out, 16)

        @block.tensor
        def _(tensor):
            for bt in range(T):
                M0 = 13 * bt
                A0 = 12 * bt
                # x tile + (first iter) weights; also guards ps1 reuse vs
                # ACT's a3 reads of tile bt-1.
                tensor.wait_ge(dma_in, 16 * n_wdma + 16 * (bt + 1))
                if bt > 0:
                    tensor.wait_ge(act_sem, A0)
                # L1: [S,N] x -> psum [512, N] over 4 o-chunks, K=S=16
                for c in range(OC):
                    mm = tensor.matmul(
                        ps1[c].ap(),
                        lhsT=w1s.ap()[:, 128 * c:128 * (c + 1)],
                        rhs=xs[bt % NBX].ap(),
                        start=True,
                        stop=True,
                    )
                    mm.then_inc(mm_sem, 1)
                # L2: guards ps2 banks vs DVE read of tile bt-1's ps4 region
                tensor.wait_ge(dve_sem, bt)
                for k in range(KC):
                    tensor.wait_ge(act_sem, A0 + k + 1)
                    for o in range(OC):
                        mm = tensor.matmul(
                            ps2[o].ap(),
                            lhsT=w2s.ap()[:, H * k + 128 * o:H * k + 128 * (o + 1)],
                            rhs=a1.ap()[:, N * k:N * (k + 1)],
                            start=(k == 0),
                            stop=(k == KC - 1),
                            skip_group_check=True,
                        )
                        if k == KC - 1:
                            mm.then_inc(mm_sem, 1)
                # L3
                for k in range(KC):
                    tensor.wait_ge(act_sem, A0 + 4 + k + 1)
                    for o in range(OC):
                        mm = tensor.matmul(
                            ps1[o].ap(),
                            lhsT=w3s.ap()[:, H * k + 128 * o:H * k + 128 * (o + 1)],
                            rhs=a2.ap()[:, N * k:N * (k + 1)],
                            start=(k == 0),
                            stop=(k == KC - 1),
                            skip_group_check=True,
                        )
                        if k == KC - 1:
                            mm.then_inc(mm_sem, 1)
                # L4: [16, N] into ps2[0] rows 0..15
                for k in range(KC):
                    tensor.wait_ge(act_sem, A0 + 8 + k + 1)
                    mm = tensor.matmul(
                        ps2[0].ap()[:S, :],
                        lhsT=w4s.ap()[:, S * k:S * (k + 1)],
                        rhs=a3.ap()[:, N * k:N * (k + 1)],
                        start=(k == 0),
                        stop=(k == KC - 1),
                        skip_group_check=True,
                    )
                    if k == KC - 1:
                        mm.then_inc(mm_sem, 1)

        @block.scalar
        def _(scalar):
            silu = mybir.ActivationFunctionType.Silu
            for bt in range(T):
                M0 = 13 * bt
                for c in range(OC):
                    scalar.wait_ge(mm_sem, M0 + c + 1)
                    scalar.activation(
                        out=a1.ap()[:, N * c:N * (c + 1)],
                        in_=ps1[c].ap(),
                        func=silu,
                        bias=b1s.ap()[:, c:c + 1],
                        scale=1.0,
                    ).then_inc(act_sem, 1)
                for o in range(OC):
                    scalar.wait_ge(mm_sem, M0 + 4 + o + 1)
                    scalar.activation(
                        out=a2.ap()[:, N * o:N * (o + 1)],
                        in_=ps2[o].ap(),
                        func=silu,
                        bias=b2s.ap()[:, o:o + 1],
                        scale=1.0,
                    ).then_inc(act_sem, 1)
                for o in range(OC):
                    scalar.wait_ge(mm_sem, M0 + 8 + o + 1)
                    scalar.activation(
                        out=a3.ap()[:, N * o:N * (o + 1)],
                        in_=ps1[o].ap(),
                        func=silu,
                        bias=b3s.ap()[:, o:o + 1],
                        scale=1.0,
                    ).then_inc(act_sem, 1)

        @block.vector
        def _(vector):
            for bt in range(T):
                vector.wait_ge(mm_sem, 13 * (bt + 1))
                if bt >= NBY:
                    vector.wait_ge(dma_out, 16 * (bt - NBY + 1))
                vector.tensor_scalar_add(
                    out=ys[bt % NBY].ap(),
                    in0=ps2[0].ap()[:S, :],
                    scalar1=b4s.ap(),
                ).then_inc(dve_sem, 1)

    return nc


def _prep_host(x, W1, b1, beta1, W2, b2, beta2, W3, b3, beta3, W4, b4):
    """Fold swish scalars into weights; build per-core input maps."""
    in_maps = []
    sp1 = np.log1p(np.exp(beta1.astype(np.float64)))
    sp2 = np.log1p(np.exp(beta2.astype(np.float64)))
    sp3 = np.log1p(np.exp(beta3.astype(np.float64)))
    c1 = 1.0 / (SWISH_DIV * sp1)
    c2 = 1.0 / (SWISH_DIV * sp2)
    c3 = 1.0 / (SWISH_DIV * sp3)
    for g in range(G):
        w1p = (sp1[g] * W1[g].astype(np.float64)).T        # [S, H]
        b1p = (sp1[g] * b1[g].astype(np.float64))          # [H]
        w2p = (sp2[g] * c1[g] * W2[g].astype(np.float64)).T  # [I, O]
        b2p = (sp2[g] * b2[g].astype(np.float64))
        w3p = (sp3[g] * c2[g] * W3[g].astype(np.float64)).T
        b3p = (sp3[g] * b3[g].astype(np.float64))
        w4p = (c3[g] * W4[g].astype(np.float64)).T          # [H, S]
        b4p = b4[g].astype(np.float64)                      # [S]
        f = np.float32
        in_maps.append({
            "xT": np.ascontiguousarray(x[:, S * g:S * (g + 1)].T),
            "w1": np.ascontiguousarray(w1p).astype(f),
            "w2": np.ascontiguousarray(w2p).astype(f),
            "w3": np.ascontiguousarray(w3p).astype(f),
            "w4": np.ascontiguousarray(w4p).astype(f),
            "b1": np.ascontiguousarray(b1p.reshape(OC, 128).T).astype(f),
            "b2": np.ascontiguousarray(b2p.reshape(OC, 128).T).astype(f),
            "b3": np.ascontiguousarray(b3p.reshape(OC, 128).T).astype(f),
            "b4": b4p.reshape(S, 1).astype(f),
        })
    return in_maps


def kernel(x, W1, b1, beta1, W2, b2, beta2, W3, b3, beta3, W4, b4,
           _trace=False, _trace_kwargs=None):
    if "nc" not in _CACHED:
        _CACHED["nc"] = build_nc()
    nc = _CACHED["nc"]
    in_maps = _prep_host(x, W1, b1, beta1, W2, b2, beta2, W3, b3, beta3, W4, b4)
    res = run_bass_kernel_spmd(
        nc, in_maps, core_ids=list(range(G)),
        trace=_trace, **(_trace_kwargs or {}),
    )
    out = np.empty((B, G * S), dtype=np.float32)
    for g in range(G):
        out[:, S * g:S * (g + 1)] = res.results[g]["y"].T
    if _trace:
        kernel._last_res = res
    return out


# revision 2
# speedup vs baseline: 1.0005x; 1.0005x over previous
"""GroupConvMLP Trainium2 kernel.

Problem: x [16384, 128] -> per-group MLP, G=8 groups of S=16 channels.
Per group g: h = x_g @ W1_g.T + b1; swish; @W2.T+b2; swish; @W3.T+b3;
swish; @W4.T+b4.  swish(x) = x*sigmoid(x*softplus(beta))/1.1.

Sharding: group-parallel, one group per NeuronCore (8 groups, 8 cores).

Device kernel (per core) is a pure matmul+Silu chain in channel-major
layout [C, B]:
  - host folds softplus(beta) and the 1/1.1 factor into the weights:
      h' = sp*h  =>  W' = sp*W, b' = sp*b and
      a = h*sigmoid(h*sp)/1.1 = silu(h')/(1.1*sp) = silu(h') * c
    with c folded into the NEXT layer's weights.
  - matmuls run as float32r (TF32-like, 1 cycle/row at N>=256).
  - activations on ScalarE: Silu(psum + bias) in one instruction.
  - B=16384 processed in 32 tiles of N=512 columns.
"""
import numpy as np

import concourse.bass as bass
import concourse.mybir as mybir
from concourse.bass_utils import run_bass_kernel_spmd

B = 16384
G = 8
S = 16
H = 512
SWISH_DIV = 1.1

N = 512            # batch columns per tile
T = B // N         # 32 tiles
KC = H // 128      # 4 contraction chunks for H-dim
OC = H // 128      # 4 output chunks for H-dim
NBX = 4            # x tile buffers
NBY = 4            # y tile buffers
LA = 2             # DMA lookahead (in-DMAs run LA tiles ahead of out-DMAs)

F32 = mybir.dt.float32
F32R = mybir.dt.float32r

_CACHED = {}


def build_nc():
    nc = bass.Bass()

    xT = nc.declare_dram_parameter("xT", [S, B], F32, isOutput=False)
    w1 = nc.declare_dram_parameter("w1", [S, H], F32, isOutput=False)
    w2 = nc.declare_dram_parameter("w2", [H, H], F32, isOutput=False)
    w3 = nc.declare_dram_parameter("w3", [H, H], F32, isOutput=False)
    w4 = nc.declare_dram_parameter("w4", [H, S], F32, isOutput=False)
    b1 = nc.declare_dram_parameter("b1", [128, OC], F32, isOutput=False)
    b2 = nc.declare_dram_parameter("b2", [128, OC], F32, isOutput=False)
    b3 = nc.declare_dram_parameter("b3", [128, OC], F32, isOutput=False)
    b4 = nc.declare_dram_parameter("b4", [S, 1], F32, isOutput=False)
    y = nc.declare_dram_parameter("y", [S, B], F32, isOutput=True)

    # SBUF
    w1s = nc.alloc_sbuf_tensor("w1s", [S, H], F32R)
    w2s = nc.alloc_sbuf_tensor("w2s", [128, KC * H], F32R)   # chunk (k,o) at 512k+128o
    w3s = nc.alloc_sbuf_tensor("w3s", [128, KC * H], F32R)
    w4s = nc.alloc_sbuf_tensor("w4s", [128, KC * S], F32R)   # chunk k at 16k
    b1s = nc.alloc_sbuf_tensor("b1s", [128, OC], F32)
    b2s = nc.alloc_sbuf_tensor("b2s", [128, OC], F32)
    b3s = nc.alloc_sbuf_tensor("b3s", [128, OC], F32)
    b4s = nc.alloc_sbuf_tensor("b4s", [S, 1], F32)
    xs = [nc.alloc_sbuf_tensor(f"xs{i}", [S, N], F32R) for i in range(NBX)]
    ys = [nc.alloc_sbuf_tensor(f"ys{i}", [S, N], F32) for i in range(NBY)]
    a1 = nc.alloc_sbuf_tensor("a1", [128, KC * N], F32R)     # k-chunk at 512k
    a2 = nc.alloc_sbuf_tensor("a2", [128, KC * N], F32R)
    a3 = nc.alloc_sbuf_tensor("a3", [128, KC * N], F32R)

    # PSUM: 8 banks
    ps1 = [nc.alloc_psum_tensor(f"ps1_{i}", [128, N], F32) for i in range(4)]
    ps2 = [nc.alloc_psum_tensor(f"ps2_{i}", [128, N], F32) for i in range(4)]

    n_wdma = 17  # weight DMAs emitted below

    with (
        nc.Block() as block,
        nc.semaphore("dma_w") as dma_w,
        nc.semaphore("dma_x0") as dma_x0,
        nc.semaphore("dma_x1") as dma_x1,
        nc.semaphore("dma_x2") as dma_x2,
        nc.semaphore("dma_x3") as dma_x3,
        nc.semaphore("dma_o0") as dma_o0,
        nc.semaphore("dma_o1") as dma_o1,
        nc.semaphore("dma_o2") as dma_o2,
        nc.semaphore("dma_o3") as dma_o3,
        nc.semaphore("mm_sem") as mm_sem,
        nc.semaphore("act_sem") as act_sem,
        nc.semaphore("dve_sem") as dve_sem,
    ):
        dma_x = [dma_x0, dma_x1, dma_x2, dma_x3]
        dma_o = [dma_o0, dma_o1, dma_o2, dma_o3]

        @block.sync
        def _(sync):
            r = F32R
            sync.dma_start(out=w1s.ap(), in_=w1.ap().bitcast(r)).then_inc(dma_w, 16)
            for k in range(KC):
                sync.dma_start(
                    out=w2s.ap()[:, k * H:(k + 1) * H],
                    in_=w2.ap()[128 * k:128 * (k + 1), :].bitcast(r),
                ).then_inc(dma_w, 16)
            for k in range(KC):
                sync.dma_start(
                    out=w3s.ap()[:, k * H:(k + 1) * H],
                    in_=w3.ap()[128 * k:128 * (k + 1), :].bitcast(r),
                ).then_inc(dma_w, 16)
            for k in range(KC):
                sync.dma_start(
                    out=w4s.ap()[:, k * S:(k + 1) * S],
                    in_=w4.ap()[128 * k:128 * (k + 1), :].bitcast(r),
                ).then_inc(dma_w, 16)
            sync.dma_start(out=b1s.ap(), in_=b1.ap()).then_inc(dma_w, 16)
            sync.dma_start(out=b2s.ap(), in_=b2.ap()).then_inc(dma_w, 16)
            sync.dma_start(out=b3s.ap(), in_=b3.ap()).then_inc(dma_w, 16)
            sync.dma_start(out=b4s.ap(), in_=b4.ap()).then_inc(dma_w, 16)

            for i in range(T + LA):
                if i < T:
                    if i >= NBX:
                        # xs[i % NBX] still being read by PE tile i-NBX L1
                        sync.wait_ge(mm_sem, 13 * (i - NBX) + 4)
                    sync.dma_start(
                        out=xs[i % NBX].ap(),
                        in_=xT.ap()[:, N * i:N * (i + 1)].bitcast(r),
                    ).then_inc(dma_x[i % NBX], 16)
                if i >= LA:
                    j = i - LA
                    sync.wait_ge(dve_sem, j + 1)
                    sync.dma_start(
                        out=y.ap()[:, N * j:N * (j + 1)],
                        in_=ys[j % NBY].ap(),
                    ).then_inc(dma_o[j % NBY], 16)

        @block.tensor
        def _(tensor):
            for bt in range(T):
                M0 = 13 * bt
                A0 = 12 * bt
                # x tile + (first iter) weights; also guards ps1 reuse vs
                # ACT's a3 reads of tile bt-1.
                if bt == 0:
                    tensor.wait_ge(dma_w, 16 * n_wdma)
                tensor.wait_ge(dma_x[bt % NBX], 16 * (bt // NBX + 1))
                if bt > 0:
                    tensor.wait_ge(act_sem, A0)
                # L1: [S,N] x -> psum [512, N] over 4 o-chunks, K=S=16
                for c in range(OC):
                    mm = tensor.matmul(
                        ps1[c].ap(),
                        lhsT=w1s.ap()[:, 128 * c:128 * (c + 1)],
                        rhs=xs[bt % NBX].ap(),
                        start=True,
                        stop=True,
                    )
                    mm.then_inc(mm_sem, 1)
                # L2: guards ps2 banks vs DVE read of tile bt-1's ps4 region
                tensor.wait_ge(dve_sem, bt)
                for k in range(KC):
                    tensor.wait_ge(act_sem, A0 + k + 1)
                    for o in range(OC):
                        mm = tensor.matmul(
                            ps2[o].ap(),
                            lhsT=w2s.ap()[:, H * k + 128 * o:H * k + 128 * (o + 1)],
                            rhs=a1.ap()[:, N * k:N * (k + 1)],
                            start=(k == 0),
                            stop=(k == KC - 1),
                            skip_group_check=True,
                        )
                        if k == KC - 1:
                            mm.then_inc(mm_sem, 1)
                # L3
                for k in range(KC):
                    tensor.wait_ge(act_sem, A0 + 4 + k + 1)
                    for o in range(OC):
                        mm = tensor.matmul(
                            ps1[o].ap(),
                            lhsT=w3s.ap()[:, H * k + 128 * o:H * k + 128 * (o + 1)],
                            rhs=a2.ap()[:, N * k:N * (k + 1)],
                            start=(k == 0),
                            stop=(k == KC - 1),
                            skip_group_check=True,
                        )
                        if k == KC - 1:
                            mm.then_inc(mm_sem, 1)
                # L4: [16, N] into ps2[0] rows 0..15
                for k in range(KC):
                    tensor.wait_ge(act_sem, A0 + 8 + k + 1)
                    mm = tensor.matmul(
                        ps2[0].ap()[:S, :],
                        lhsT=w4s.ap()[:, S * k:S * (k + 1)],
                        rhs=a3.ap()[:, N * k:N * (k + 1)],
                        start=(k == 0),
                        stop=(k == KC - 1),
                        skip_group_check=True,
                    )
                    if k == KC - 1:
                        mm.then_inc(mm_sem, 1)

        @block.scalar
        def _(scalar):
            silu = mybir.ActivationFunctionType.Silu
            for bt in range(T):
                M0 = 13 * bt
                for c in range(OC):
                    scalar.wait_ge(mm_sem, M0 + c + 1)
                    scalar.activation(
                        out=a1.ap()[:, N * c:N * (c + 1)],
                        in_=ps1[c].ap(),
                        func=silu,
                        bias=b1s.ap()[:, c:c + 1],
                        scale=1.0,
                    ).then_inc(act_sem, 1)
                for o in range(OC):
                    scalar.wait_ge(mm_sem, M0 + 4 + o + 1)
                    scalar.activation(
                        out=a2.ap()[:, N * o:N * (o + 1)],
                        in_=ps2[o].ap(),
                        func=silu,
                        bias=b2s.ap()[:, o:o + 1],
                        scale=1.0,
                    ).then_inc(act_sem, 1)
                for o in range(OC):
                    scalar.wait_ge(mm_sem, M0 + 8 + o + 1)
                    scalar.activation(
                        out=a3.ap()[:, N * o:N * (o + 1)],
                        in_=ps1[o].ap(),
                        func=silu,
                        bias=b3s.ap()[:, o:o + 1],
                        scale=1.0,
                    ).then_inc(act_sem, 1)

        @block.vector
        def _(vector):
            for bt in range(T):
                vector.wait_ge(mm_sem, 13 * (bt + 1))
                if bt >= NBY:
                    vector.wait_ge(dma_o[bt % NBY], 16 * (bt // NBY))
                vector.tensor_scalar_add(
                    out=ys[bt % NBY].ap(),
                    in0=ps2[0].ap()[:S, :],
                    scalar1=b4s.ap(),
                ).then_inc(dve_sem, 1)

    return nc


def _prep_host(x, W1, b1, beta1, W2, b2, beta2, W3, b3, beta3, W4, b4):
    """Fold swish scalars into weights; build per-core input maps."""
    in_maps = []
    sp1 = np.log1p(np.exp(beta1.astype(np.float64)))
    sp2 = np.log1p(np.exp(beta2.astype(np.float64)))
    sp3 = np.log1p(np.exp(beta3.astype(np.float64)))
    c1 = 1.0 / (SWISH_DIV * sp1)
    c2 = 1.0 / (SWISH_DIV * sp2)
    c3 = 1.0 / (SWISH_DIV * sp3)
    for g in range(G):
        w1p = (sp1[g] * W1[g].astype(np.float64)).T        # [S, H]
        b1p = (sp1[g] * b1[g].astype(np.float64))          # [H]
        w2p = (sp2[g] * c1[g] * W2[g].astype(np.float64)).T  # [I, O]
        b2p = (sp2[g] * b2[g].astype(np.float64))
        w3p = (sp3[g] * c2[g] * W3[g].astype(np.float64)).T
        b3p = (sp3[g] * b3[g].astype(np.float64))
        w4p = (c3[g] * W4[g].astype(np.float64)).T          # [H, S]
        b4p = b4[g].astype(np.float64)                      # [S]
        f = np.float32
        in_maps.append({
            "xT": np.ascontiguousarray(x[:, S * g:S * (g + 1)].T),
            "w1": np.ascontiguousarray(w1p).astype(f),
            "w2": np.ascontiguousarray(w2p).astype(f),
            "w3": np.ascontiguousarray(w3p).astype(f),
            "w4": np.ascontiguousarray(w4p).astype(f),
            "b1": np.ascontiguousarray(b1p.reshape(OC, 128).T).astype(f),
            "b2": np.ascontiguousarray(b2p.reshape(OC, 128).T).astype(f),
            "b3": np.ascontiguousarray(b3p.reshape(OC, 128).T).astype(f),
            "b4": b4p.reshape(S, 1).astype(f),
        })
    return in_maps


def kernel(x, W1, b1, beta1, W2, b2, beta2, W3, b3, beta3, W4, b4,
           _trace=False, _trace_kwargs=None):
    if "nc" not in _CACHED:
        _CACHED["nc"] = build_nc()
    nc = _CACHED["nc"]
    in_maps = _prep_host(x, W1, b1, beta1, W2, b2, beta2, W3, b3, beta3, W4, b4)
    res = run_bass_kernel_spmd(
        nc, in_maps, core_ids=list(range(G)),
        trace=_trace, **(_trace_kwargs or {}),
    )
    out = np.empty((B, G * S), dtype=np.float32)
    for g in range(G):
        out[:, S * g:S * (g + 1)] = res.results[g]["y"].T
    if _trace:
        kernel._last_res = res
    return out


# revision 3
# speedup vs baseline: 1.0934x; 1.0928x over previous
"""GroupConvMLP Trainium2 kernel.

Problem: x [16384, 128] -> per-group MLP, G=8 groups of S=16 channels.
Per group g: h = x_g @ W1_g.T + b1; swish; @W2.T+b2; swish; @W3.T+b3;
swish; @W4.T+b4.  swish(x) = x*sigmoid(x*softplus(beta))/1.1.

Sharding: group-parallel, one group per NeuronCore (8 groups, 8 cores).

Device kernel (per core) is a pure matmul+Silu chain in channel-major
layout [C, B]:
  - host folds softplus(beta) and the 1/1.1 factor into the weights:
      h' = sp*h  =>  W' = sp*W, b' = sp*b and
      a = h*sigmoid(h*sp)/1.1 = silu(h')/(1.1*sp) = silu(h') * c
    with c folded into the NEXT layer's weights.
  - matmuls run as float32r (TF32-like, 1 cycle/row at N>=256).
  - activations on ScalarE: Silu(psum + bias) in one instruction.
  - B=16384 processed in 32 tiles of N=512 columns.
"""
import numpy as np

import concourse.bass as bass
import concourse.mybir as mybir
from concourse.bass_utils import run_bass_kernel_spmd

B = 16384
G = 8
S = 16
H = 512
SWISH_DIV = 1.1

N = 512            # batch columns per tile
T = B // N         # 32 tiles
KC = H // 128      # 4 contraction chunks for H-dim
OC = H // 128      # 4 output chunks for H-dim
NBX = 4            # x tile buffers
NBY = 4            # y tile buffers
LA = 2             # DMA lookahead (in-DMAs run LA tiles ahead of out-DMAs)

F32 = mybir.dt.float32
F32R = mybir.dt.float32r
BF16 = mybir.dt.bfloat16

_CACHED = {}


def build_nc():
    nc = bass.Bass()

    xT = nc.declare_dram_parameter("xT", [S, B], BF16, isOutput=False)
    w1 = nc.declare_dram_parameter("w1", [S, H], BF16, isOutput=False)
    w2 = nc.declare_dram_parameter("w2", [H, H], BF16, isOutput=False)
    w3 = nc.declare_dram_parameter("w3", [H, H], BF16, isOutput=False)
    w4 = nc.declare_dram_parameter("w4", [H, S], BF16, isOutput=False)
    b1 = nc.declare_dram_parameter("b1", [128, OC], F32, isOutput=False)
    b2 = nc.declare_dram_parameter("b2", [128, OC], F32, isOutput=False)
    b3 = nc.declare_dram_parameter("b3", [128, OC], F32, isOutput=False)
    b4 = nc.declare_dram_parameter("b4", [S, 1], F32, isOutput=False)
    y = nc.declare_dram_parameter("y", [S, B], F32, isOutput=True)

    # SBUF
    w1s = nc.alloc_sbuf_tensor("w1s", [S, H], BF16)
    w2s = nc.alloc_sbuf_tensor("w2s", [128, KC * H], BF16)   # chunk (k,o) at 512k+128o
    w3s = nc.alloc_sbuf_tensor("w3s", [128, KC * H], BF16)
    w4s = nc.alloc_sbuf_tensor("w4s", [128, KC * S], BF16)   # chunk k at 16k
    b1s = nc.alloc_sbuf_tensor("b1s", [128, OC], F32)
    b2s = nc.alloc_sbuf_tensor("b2s", [128, OC], F32)
    b3s = nc.alloc_sbuf_tensor("b3s", [128, OC], F32)
    b4s = nc.alloc_sbuf_tensor("b4s", [S, 1], F32)
    xs = [nc.alloc_sbuf_tensor(f"xs{i}", [S, N], BF16) for i in range(NBX)]
    ys = [nc.alloc_sbuf_tensor(f"ys{i}", [S, N], F32) for i in range(NBY)]
    a1 = nc.alloc_sbuf_tensor("a1", [128, KC * N], BF16)     # k-chunk at 512k
    a2 = nc.alloc_sbuf_tensor("a2", [128, KC * N], BF16)
    a3 = nc.alloc_sbuf_tensor("a3", [128, KC * N], BF16)

    # PSUM: 8 banks
    ps1 = [nc.alloc_psum_tensor(f"ps1_{i}", [128, N], F32) for i in range(4)]
    ps2 = [nc.alloc_psum_tensor(f"ps2_{i}", [128, N], F32) for i in range(4)]

    n_wdma = 17  # weight DMAs emitted below

    with (
        nc.Block() as block,
        nc.semaphore("dma_w") as dma_w,
        nc.semaphore("dma_x0") as dma_x0,
        nc.semaphore("dma_x1") as dma_x1,
        nc.semaphore("dma_x2") as dma_x2,
        nc.semaphore("dma_x3") as dma_x3,
        nc.semaphore("dma_o0") as dma_o0,
        nc.semaphore("dma_o1") as dma_o1,
        nc.semaphore("dma_o2") as dma_o2,
        nc.semaphore("dma_o3") as dma_o3,
        nc.semaphore("mm_sem") as mm_sem,
        nc.semaphore("act_sem") as act_sem,
        nc.semaphore("dve_sem") as dve_sem,
    ):
        dma_x = [dma_x0, dma_x1, dma_x2, dma_x3]
        dma_o = [dma_o0, dma_o1, dma_o2, dma_o3]

        @block.sync
        def _(sync):
            sync.dma_start(out=w1s.ap(), in_=w1.ap()).then_inc(dma_w, 16)
            for k in range(KC):
                sync.dma_start(
                    out=w2s.ap()[:, k * H:(k + 1) * H],
                    in_=w2.ap()[128 * k:128 * (k + 1), :],
                ).then_inc(dma_w, 16)
            for k in range(KC):
                sync.dma_start(
                    out=w3s.ap()[:, k * H:(k + 1) * H],
                    in_=w3.ap()[128 * k:128 * (k + 1), :],
                ).then_inc(dma_w, 16)
            for k in range(KC):
                sync.dma_start(
                    out=w4s.ap()[:, k * S:(k + 1) * S],
                    in_=w4.ap()[128 * k:128 * (k + 1), :],
                ).then_inc(dma_w, 16)
            sync.dma_start(out=b1s.ap(), in_=b1.ap()).then_inc(dma_w, 16)
            sync.dma_start(out=b2s.ap(), in_=b2.ap()).then_inc(dma_w, 16)
            sync.dma_start(out=b3s.ap(), in_=b3.ap()).then_inc(dma_w, 16)
            sync.dma_start(out=b4s.ap(), in_=b4.ap()).then_inc(dma_w, 16)

            for i in range(T + LA):
                if i < T:
                    if i >= NBX:
                        # xs[i % NBX] still being read by PE tile i-NBX L1
                        sync.wait_ge(mm_sem, 13 * (i - NBX) + 4)
                    sync.dma_start(
                        out=xs[i % NBX].ap(),
                        in_=xT.ap()[:, N * i:N * (i + 1)],
                    ).then_inc(dma_x[i % NBX], 16)
                if i >= LA:
                    j = i - LA
                    sync.wait_ge(dve_sem, j + 1)
                    sync.dma_start(
                        out=y.ap()[:, N * j:N * (j + 1)],
                        in_=ys[j % NBY].ap(),
                    ).then_inc(dma_o[j % NBY], 16)

        @block.tensor
        def _(tensor):
            for bt in range(T):
                M0 = 13 * bt
                A0 = 12 * bt
                # x tile + (first iter) weights; also guards ps1 reuse vs
                # ACT's a3 reads of tile bt-1.
                if bt == 0:
                    tensor.wait_ge(dma_w, 16 * n_wdma)
                tensor.wait_ge(dma_x[bt % NBX], 16 * (bt // NBX + 1))
                if bt > 0:
                    tensor.wait_ge(act_sem, A0)
                # L1: [S,N] x -> psum [512, N] over 4 o-chunks, K=S=16
                for c in range(OC):
                    mm = tensor.matmul(
                        ps1[c].ap(),
                        lhsT=w1s.ap()[:, 128 * c:128 * (c + 1)],
                        rhs=xs[bt % NBX].ap(),
                        start=True,
                        stop=True,
                    )
                    mm.then_inc(mm_sem, 1)
                # L2: guards ps2 banks vs DVE read of tile bt-1's ps4 region
                tensor.wait_ge(dve_sem, bt)
                for k in range(KC):
                    tensor.wait_ge(act_sem, A0 + k + 1)
                    for o in range(OC):
                        mm = tensor.matmul(
                            ps2[o].ap(),
                            lhsT=w2s.ap()[:, H * k + 128 * o:H * k + 128 * (o + 1)],
                            rhs=a1.ap()[:, N * k:N * (k + 1)],
                            start=(k == 0),
                            stop=(k == KC - 1),
                            skip_group_check=True,
                        )
                        if k == KC - 1:
                            mm.then_inc(mm_sem, 1)
                # L3
                for k in range(KC):
                    tensor.wait_ge(act_sem, A0 + 4 + k + 1)
                    for o in range(OC):
                        mm = tensor.matmul(
                            ps1[o].ap(),
                            lhsT=w3s.ap()[:, H * k + 128 * o:H * k + 128 * (o + 1)],
                            rhs=a2.ap()[:, N * k:N * (k + 1)],
                            start=(k == 0),
                            stop=(k == KC - 1),
                            skip_group_check=True,
                        )
                        if k == KC - 1:
                            mm.then_inc(mm_sem, 1)
                # L4: [16, N] into ps2[0] rows 0..15
                for k in range(KC):
                    tensor.wait_ge(act_sem, A0 + 8 + k + 1)
                    mm = tensor.matmul(
                        ps2[0].ap()[:S, :],
                        lhsT=w4s.ap()[:, S * k:S * (k + 1)],
                        rhs=a3.ap()[:, N * k:N * (k + 1)],
                        start=(k == 0),
                        stop=(k == KC - 1),
                        skip_group_check=True,
                    )
                    if k == KC - 1:
                        mm.then_inc(mm_sem, 1)

        @block.scalar
        def _(scalar):
            silu = mybir.ActivationFunctionType.Silu
            for bt in range(T):
                M0 = 13 * bt
                for c in range(OC):
                    scalar.wait_ge(mm_sem, M0 + c + 1)
                    scalar.activation(
                        out=a1.ap()[:, N * c:N * (c + 1)],
                        in_=ps1[c].ap(),
                        func=silu,
                        bias=b1s.ap()[:, c:c + 1],
                        scale=1.0,
                    ).then_inc(act_sem, 1)
                for o in range(OC):
                    scalar.wait_ge(mm_sem, M0 + 4 + o + 1)
                    scalar.activation(
                        out=a2.ap()[:, N * o:N * (o + 1)],
                        in_=ps2[o].ap(),
                        func=silu,
                        bias=b2s.ap()[:, o:o + 1],
                        scale=1.0,
                    ).then_inc(act_sem, 1)
                for o in range(OC):
                    scalar.wait_ge(mm_sem, M0 + 8 + o + 1)
                    scalar.activation(
                        out=a3.ap()[:, N * o:N * (o + 1)],
                        in_=ps1[o].ap(),
                        func=silu,
                        bias=b3s.ap()[:, o:o + 1],
                        scale=1.0,
                    ).then_inc(act_sem, 1)

        @block.vector
        def _(vector):
            for bt in range(T):
                vector.wait_ge(mm_sem, 13 * (bt + 1))
                if bt >= NBY:
                    vector.wait_ge(dma_o[bt % NBY], 16 * (bt // NBY))
                vector.tensor_scalar_add(
                    out=ys[bt % NBY].ap(),
                    in0=ps2[0].ap()[:S, :],
                    scalar1=b4s.ap(),
                ).then_inc(dve_sem, 1)

    return nc


def _prep_host(x, W1, b1, beta1, W2, b2, beta2, W3, b3, beta3, W4, b4):
    """Fold swish scalars into weights; build per-core input maps."""
    in_maps = []
    sp1 = np.log1p(np.exp(beta1.astype(np.float64)))
    sp2 = np.log1p(np.exp(beta2.astype(np.float64)))
    sp3 = np.log1p(np.exp(beta3.astype(np.float64)))
    c1 = 1.0 / (SWISH_DIV * sp1)
    c2 = 1.0 / (SWISH_DIV * sp2)
    c3 = 1.0 / (SWISH_DIV * sp3)
    for g in range(G):
        w1p = (sp1[g] * W1[g].astype(np.float64)).T        # [S, H]
        b1p = (sp1[g] * b1[g].astype(np.float64))          # [H]
        w2p = (sp2[g] * c1[g] * W2[g].astype(np.float64)).T  # [I, O]
        b2p = (sp2[g] * b2[g].astype(np.float64))
        w3p = (sp3[g] * c2[g] * W3[g].astype(np.float64)).T
        b3p = (sp3[g] * b3[g].astype(np.float64))
        w4p = (c3[g] * W4[g].astype(np.float64)).T          # [H, S]
        b4p = b4[g].astype(np.float64)                      # [S]
        f = np.float32
        import ml_dtypes
        bf = ml_dtypes.bfloat16
        in_maps.append({
            "xT": np.ascontiguousarray(x[:, S * g:S * (g + 1)].T).astype(bf),
            "w1": np.ascontiguousarray(w1p).astype(bf),
            "w2": np.ascontiguousarray(w2p).astype(bf),
            "w3": np.ascontiguousarray(w3p).astype(bf),
            "w4": np.ascontiguousarray(w4p).astype(bf),
            "b1": np.ascontiguousarray(b1p.reshape(OC, 128).T).astype(f),
            "b2": np.ascontiguousarray(b2p.reshape(OC, 128).T).astype(f),
            "b3": np.ascontiguousarray(b3p.reshape(OC, 128).T).astype(f),
            "b4": b4p.reshape(S, 1).astype(f),
        })
    return in_maps


def kernel(x, W1, b1, beta1, W2, b2, beta2, W3, b3, beta3, W4, b4,
           _trace=False, _trace_kwargs=None):
    if "nc" not in _CACHED:
        _CACHED["nc"] = build_nc()
    nc = _CACHED["nc"]
    in_maps = _prep_host(x, W1, b1, beta1, W2, b2, beta2, W3, b3, beta3, W4, b4)
    res = run_bass_kernel_spmd(
        nc, in_maps, core_ids=list(range(G)),
        trace=_trace, **(_trace_kwargs or {}),
    )
    out = np.empty((B, G * S), dtype=np.float32)
    for g in range(G):
        out[:, S * g:S * (g + 1)] = res.results[g]["y"].T
    if _trace:
        kernel._last_res = res
    return out
